# revision 9
# baseline (speedup 1.0000x reference)
"""Distributed kNN-classifier kernel for Trainium2 (8 NeuronCores).

Strategy (classic distributed kNN, column-sharded):
  - distances [2048, 100000] f32 are sharded along the prototype (column)
    dim: core c gets columns [c*12500, (c+1)*12500).
  - On device, a 3-engine pipeline per core keeps the DMA bus (the
    roofline resource: 102.4 MB/core at ~360 GB/s) 100% busy:
      SP    streams input chunks of [128, 2500] f32 (16 row-tiles x 5
            column-chunks; the last tile's tail is split 1200/800/500 to
            shorten the pipeline drain) through an 8-slot SBUF ring.
      DVE   grouped-min-reduces each chunk (groups of 100 columns,
            negated f32 output) into 125 group minima per row-tile,
            then selects the 24 groups with the smallest minima via 3
            rounds of max8 / max_index / match_replace.  Aggregate DVE
            work is ~16.4 us per 17.8 us of tile DMA, so the DMA bus
            stays the bottleneck; only the last tile's selection sits in
            the pipeline drain.
      Act   DMAs each finished row-tile's 24 selected group ids
            [128, 24] u16 out, overlapped with the input stream.
  - Host: each group id maps to 100 consecutive columns.  The 8*24
    groups per row (19200 candidate columns) are gathered from the
    input, reduced to the exact global top-16 by (value, column-index)
    lexicographic order (bit-exact vs jax.lax.top_k tie semantics),
    labels looked up, and the mode-with-smallest-label vote computed
    exactly as the reference does.

Exactness argument: an element of per-core rank r lives in a group whose
min is among the r smallest element values, hence among the r
lexicographically-smallest group minima; selecting 24 groups therefore
covers every element of per-core rank <= 24, which covers the global
top-16 plus any realistic tie multiplicity at the boundary.
"""

import os
import sys

import numpy as np

sys.path.insert(0, "/opt/trn_rl_repo")

import concourse.bass as bass
import concourse.mybir as mybir
from concourse.bass_utils import run_bass_kernel_spmd

R = 2048          # rows (batch)
N = 100000        # prototypes (columns)
NC = 8            # cores
S = N // NC       # 12500 columns per core
P = 128           # partitions
NT = R // P       # 16 row-tiles
W = 2500          # columns per DMA chunk
CH = S // W       # 5 chunks per row-tile
G = 100           # columns per group
NG = S // G       # 125 groups per row-tile
NSEL = 24         # groups selected per row per core (3 rounds of max8)
K = 16
NUM_CLASSES = 100

NBUFX = 8         # input-chunk ring slots (8 * 10 KB/partition)

# Per-tile chunk lists: (col_offset, width).  Groups are 100 consecutive
# columns regardless of chunking, so chunk boundaries only need to be
# multiples of 100.  The last tile's final 2500 columns are split
# 1200/800/500 so the reduce after the final DMA is short.
_STD = [(c * W, W) for c in range(CH)]
_LAST = _STD[:4] + [(10000, 1200), (11200, 800), (12000, 500)]
CHUNKS = [
    (t, off, width)
    for t in range(NT)
    for (off, width) in (_LAST if t == NT - 1 else _STD)
]
NCH = len(CHUNKS)

_CACHE = {}


def build_nc():
    """Raw-Bass SPMD program. Engine pipeline:

    SP -(dma_sem)-> DVE reduce+select -(sel_sem)-> Act output-DMA.
    red_sem releases x-ring slots back to SP.  DVE write->read pairs
    need explicit drain()s (DVE writes retire ~8 pipe stages after the
    next instruction's reads issue; read->write pairs are safe).
    """
    nc = bass.Bass()
    din = nc.declare_dram_parameter("d", [R, S], mybir.dt.float32, isOutput=False)
    gout = nc.declare_dram_parameter("gidx", [R, NSEL], mybir.dt.uint16, isOutput=True)

    from contextlib import ExitStack

    with ExitStack() as ctx:
        x = ctx.enter_context(nc.sbuf_tensor([P, NBUFX * W], mybir.dt.float32))
        gneg = ctx.enter_context(nc.sbuf_tensor([P, 2 * NG], mybir.dt.float32))
        m8 = ctx.enter_context(nc.sbuf_tensor([P, 8], mybir.dt.float32))
        gidx_all = ctx.enter_context(
            nc.sbuf_tensor([P, NT * NSEL], mybir.dt.uint16)
        )
        # One DMA-completion semaphore per x-ring slot.  A dma_start's 16
        # increments are per-DMA-engine completions, so increments from
        # overlapping chunk DMAs interleave and a single cumulative
        # semaphore cannot tell "chunk k fully landed".  With one sem per
        # slot, chunk k and the next user of its sem (chunk k+NBUFX) are
        # serialized by the slot-reuse wait (SP only issues chunk k+NBUFX
        # after DVE consumed chunk k), so each wait is exact.
        dsem = [
            ctx.enter_context(nc.semaphore(f"dma_sem{j}")) for j in range(NBUFX)
        ]
        red_sem = ctx.enter_context(nc.semaphore("red_sem"))
        sel_sem = ctx.enter_context(nc.semaphore("sel_sem"))
        out_sem = ctx.enter_context(nc.semaphore("out_sem"))
        block = ctx.enter_context(nc.Block())

        @block.sync
        def _(sync):
            for k, (t, off, width) in enumerate(CHUNKS):
                if k >= NBUFX:
                    # slot's previous chunk fully consumed by DVE's reduce
                    sync.wait_ge(red_sem, k - NBUFX + 1)
                s = k % NBUFX
                sync.dma_start(
                    out=x[:, s * W : s * W + width],
                    in_=din[t * P : (t + 1) * P, off : off + width],
                ).then_inc(dsem[s], 16)

        @block.scalar
        def _(scalar):
            for t in range(NT):
                scalar.wait_ge(sel_sem, t + 1)
                scalar.dma_start(
                    out=gout[t * P : (t + 1) * P, :],
                    in_=gidx_all[:, t * NSEL : (t + 1) * NSEL],
                ).then_inc(out_sem, 16)
            scalar.wait_ge(out_sem, 16 * NT)

        @block.vector
        def _(vector):
            for k, (t, off, width) in enumerate(CHUNKS):
                par = t % 2
                s = k % NBUFX
                vector.wait_ge(dsem[s], 16 * (k // NBUFX + 1))
                # gneg[p, g] = -min over group = max over group of -x
                nc.vector.tensor_reduce(
                    out=gneg[
                        :, par * NG + off // G : par * NG + (off + width) // G
                    ],
                    in_=x[:, s * W : s * W + width].rearrange(
                        "p (g e) -> p g e", e=G
                    ),
                    axis=mybir.AxisListType.X,
                    op=mybir.AluOpType.min,
                    negate=True,
                ).then_inc(red_sem, 1)
                if off + width == S:
                    gv = gneg[:, par * NG : (par + 1) * NG]
                    nc.vector.drain()
                    for r in range(NSEL // 8):
                        nc.vector.max(out=m8[:], in_=gv)
                        nc.vector.drain()
                        nc.vector.max_index(
                            out=gidx_all[
                                :, t * NSEL + r * 8 : t * NSEL + (r + 1) * 8
                            ],
                            in_max=m8[:],
                            in_values=gv,
                        )
                        if r < NSEL // 8 - 1:
                            nc.vector.match_replace(
                                out=gv,
                                in_to_replace=m8[:],
                                in_values=gv,
                                imm_value=-3.0e38,
                            )
                            nc.vector.drain()
                    nc.vector.drain().then_inc(sel_sem, 1)

    return nc


def _sortable_u32(vals_f32):
    b = vals_f32.view(np.uint32)
    return np.where(b & 0x80000000, ~b, b | np.uint32(0x80000000)).astype(np.uint32)


def host_finish(g_idx_all, d, labels):
    """g_idx_all: [NC, R, NSEL] selected group ids. Returns winning labels [R]."""
    g = g_idx_all.transpose(1, 0, 2).astype(np.int32)  # [R, NC, NSEL]
    cols = (
        g[:, :, :, None] * G
        + np.arange(G, dtype=np.int32)[None, None, None, :]
        + (np.arange(NC, dtype=np.int32) * S)[None, :, None, None]
    ).reshape(R, -1)
    vals = np.take_along_axis(d, cols, axis=1)
    key = (_sortable_u32(vals).astype(np.uint64) << np.uint64(17)) | cols.astype(
        np.uint64
    )
    key = np.partition(key, K - 1, axis=1)[:, :K]
    key.sort(axis=1)
    top_cols = (key[:, :K] & np.uint64(0x1FFFF)).astype(np.int64)
    gathered = labels[top_cols]  # [R, K]
    eq = gathered[:, :, None] == gathered[:, None, :]
    counts = eq.sum(axis=-1)
    score = counts.astype(np.int64) * (NUM_CLASSES + 1) - gathered
    idx = np.argmax(score, axis=1)
    return np.take_along_axis(gathered, idx[:, None], axis=1)[:, 0]


def run_device(d, trace=False):
    if "nc" not in _CACHE:
        _CACHE["nc"] = build_nc()
    nc = _CACHE["nc"]
    in_maps = [
        {"d": np.ascontiguousarray(d[:, c * S : (c + 1) * S])} for c in range(NC)
    ]
    res = run_bass_kernel_spmd(nc, in_maps, list(range(NC)), trace=trace)
    g_idx_all = np.stack(
        [np.asarray(res.results[c]["gidx"]).astype(np.int64) for c in range(NC)]
    )
    return g_idx_all, res


def kernel(distances, labels):
    d = np.ascontiguousarray(np.asarray(distances, dtype=np.float32))
    lab = np.asarray(labels)
    g_idx_all, _ = run_device(d)
    out = host_finish(g_idx_all, d, lab.astype(np.int64))
    return out.astype(lab.dtype)


# revision 12
# speedup vs baseline: 1.0079x; 1.0079x over previous
"""Distributed kNN-classifier kernel for Trainium2 (8 NeuronCores).

Strategy (classic distributed kNN, column-sharded):
  - distances [2048, 100000] f32 are sharded along the prototype (column)
    dim: core c gets columns [c*12500, (c+1)*12500).
  - On device, a 3-engine pipeline per core keeps the DMA bus (the
    roofline resource: 102.4 MB/core at ~360 GB/s) 100% busy:
      SP    streams input chunks of [128, 2500] f32 (16 row-tiles x 5
            column-chunks; the last tile's tail is split 1200/800/500 to
            shorten the pipeline drain) through an 8-slot SBUF ring.
      DVE   grouped-min-reduces each chunk (groups of 100 columns,
            negated f32 output) into 125 group minima per row-tile,
            then selects the 24 groups with the smallest minima via 3
            rounds of max8 / max_index / match_replace.  Aggregate DVE
            work is ~16.4 us per 17.8 us of tile DMA, so the DMA bus
            stays the bottleneck; only the last tile's selection sits in
            the pipeline drain.
      Act   DMAs each finished row-tile's 24 selected group ids
            [128, 24] u16 out, overlapped with the input stream.
  - Host: each group id maps to 100 consecutive columns.  The 8*24
    groups per row (19200 candidate columns) are gathered from the
    input, reduced to the exact global top-16 by (value, column-index)
    lexicographic order (bit-exact vs jax.lax.top_k tie semantics),
    labels looked up, and the mode-with-smallest-label vote computed
    exactly as the reference does.

Exactness argument: an element of per-core rank r lives in a group whose
min is among the r smallest element values, hence among the r
lexicographically-smallest group minima; selecting 24 groups therefore
covers every element of per-core rank <= 24, which covers the global
top-16 plus any realistic tie multiplicity at the boundary.
"""

import os
import sys

import numpy as np

sys.path.insert(0, "/opt/trn_rl_repo")

import concourse.bass as bass
import concourse.mybir as mybir
from concourse.bass_utils import run_bass_kernel_spmd

R = 2048          # rows (batch)
N = 100000        # prototypes (columns)
NC = 8            # cores
S = N // NC       # 12500 columns per core
P = 128           # partitions
NT = R // P       # 16 row-tiles
W = 2500          # columns per DMA chunk
CH = S // W       # 5 chunks per row-tile
G = 100           # columns per group
NG = S // G       # 125 groups per row-tile
NSEL = 24         # groups selected per row per core (3 rounds of max8)
K = 16
NUM_CLASSES = 100

NBUFX = 8         # input-chunk ring slots (8 * 10 KB/partition)

# Per-tile chunk lists: (col_offset, width).  Groups are 100 consecutive
# columns regardless of chunking, so chunk boundaries only need to be
# multiples of 100.  The last tile only streams columns [0, 10500): its
# final NG-NG_LAST=20 groups are force-included as candidates on the
# host instead, and the streamed tail is split 300/200 — both so the
# DMA+reduce+select chain after the final transfer is as short as
# possible (it is the pipeline drain of the whole kernel).
_STD = [(c * W, W) for c in range(CH)]
_LAST = _STD[:4] + [(10000, 300), (10300, 200)]
CHUNKS = [
    (t, off, width)
    for t in range(NT)
    for (off, width) in (_LAST if t == NT - 1 else _STD)
]
NCH = len(CHUNKS)
S_LAST = 10500    # columns streamed for the last tile
NG_LAST = S_LAST // G  # 105 selectable groups for the last tile

_CACHE = {}


def build_nc():
    """Raw-Bass SPMD program. Engine pipeline:

    SP -(dma_sem)-> DVE reduce+select -(sel_sem)-> Act output-DMA.
    red_sem releases x-ring slots back to SP.  DVE write->read pairs
    need explicit drain()s (DVE writes retire ~8 pipe stages after the
    next instruction's reads issue; read->write pairs are safe).
    """
    nc = bass.Bass()
    din = nc.declare_dram_parameter("d", [R, S], mybir.dt.float32, isOutput=False)
    gout = nc.declare_dram_parameter("gidx", [R, NSEL], mybir.dt.uint16, isOutput=True)

    from contextlib import ExitStack

    with ExitStack() as ctx:
        x = ctx.enter_context(nc.sbuf_tensor([P, NBUFX * W], mybir.dt.float32))
        gneg = ctx.enter_context(nc.sbuf_tensor([P, 2 * NG], mybir.dt.float32))
        m8 = ctx.enter_context(nc.sbuf_tensor([P, 8], mybir.dt.float32))
        gidx_all = ctx.enter_context(
            nc.sbuf_tensor([P, NT * NSEL], mybir.dt.uint16)
        )
        # One DMA-completion semaphore per x-ring slot.  A dma_start's 16
        # increments are per-DMA-engine completions, so increments from
        # overlapping chunk DMAs interleave and a single cumulative
        # semaphore cannot tell "chunk k fully landed".  With one sem per
        # slot, chunk k and the next user of its sem (chunk k+NBUFX) are
        # serialized by the slot-reuse wait (SP only issues chunk k+NBUFX
        # after DVE consumed chunk k), so each wait is exact.
        dsem = [
            ctx.enter_context(nc.semaphore(f"dma_sem{j}")) for j in range(NBUFX)
        ]
        red_sem = ctx.enter_context(nc.semaphore("red_sem"))
        sel_sem = ctx.enter_context(nc.semaphore("sel_sem"))
        out_sem = ctx.enter_context(nc.semaphore("out_sem"))
        block = ctx.enter_context(nc.Block())

        @block.sync
        def _(sync):
            for k, (t, off, width) in enumerate(CHUNKS):
                if k >= NBUFX:
                    # slot's previous chunk fully consumed by DVE's reduce
                    sync.wait_ge(red_sem, k - NBUFX + 1)
                s = k % NBUFX
                sync.dma_start(
                    out=x[:, s * W : s * W + width],
                    in_=din[t * P : (t + 1) * P, off : off + width],
                ).then_inc(dsem[s], 16)

        @block.scalar
        def _(scalar):
            for t in range(NT):
                scalar.wait_ge(sel_sem, t + 1)
                scalar.dma_start(
                    out=gout[t * P : (t + 1) * P, :],
                    in_=gidx_all[:, t * NSEL : (t + 1) * NSEL],
                ).then_inc(out_sem, 16)
            scalar.wait_ge(out_sem, 16 * NT)

        @block.vector
        def _(vector):
            for k, (t, off, width) in enumerate(CHUNKS):
                par = t % 2
                s = k % NBUFX
                vector.wait_ge(dsem[s], 16 * (k // NBUFX + 1))
                # gneg[p, g] = -min over group = max over group of -x
                nc.vector.tensor_reduce(
                    out=gneg[
                        :, par * NG + off // G : par * NG + (off + width) // G
                    ],
                    in_=x[:, s * W : s * W + width].rearrange(
                        "p (g e) -> p g e", e=G
                    ),
                    axis=mybir.AxisListType.X,
                    op=mybir.AluOpType.min,
                    negate=True,
                ).then_inc(red_sem, 1)
                tile_end = S_LAST if t == NT - 1 else S
                if off + width == tile_end:
                    ng = NG_LAST if t == NT - 1 else NG
                    gv = gneg[:, par * NG : par * NG + ng]
                    nc.vector.drain()
                    for r in range(NSEL // 8):
                        nc.vector.max(out=m8[:], in_=gv)
                        nc.vector.drain()
                        nc.vector.max_index(
                            out=gidx_all[
                                :, t * NSEL + r * 8 : t * NSEL + (r + 1) * 8
                            ],
                            in_max=m8[:],
                            in_values=gv,
                        )
                        if r < NSEL // 8 - 1:
                            nc.vector.match_replace(
                                out=gv,
                                in_to_replace=m8[:],
                                in_values=gv,
                                imm_value=-3.0e38,
                            )
                            nc.vector.drain()
                    nc.vector.drain().then_inc(sel_sem, 1)

    return nc


def _sortable_u32(vals_f32):
    b = vals_f32.view(np.uint32)
    return np.where(b & 0x80000000, ~b, b | np.uint32(0x80000000)).astype(np.uint32)


def _vote(d_rows, cols, labels):
    """Exact top-K + mode vote for a row batch given candidate columns
    (cols must be duplicate-free per row)."""
    vals = np.take_along_axis(d_rows, cols, axis=1)
    key = (_sortable_u32(vals).astype(np.uint64) << np.uint64(17)) | cols.astype(
        np.uint64
    )
    key = np.partition(key, K - 1, axis=1)[:, :K]
    key.sort(axis=1)
    top_cols = (key[:, :K] & np.uint64(0x1FFFF)).astype(np.int64)
    gathered = labels[top_cols]  # [rows, K]
    eq = gathered[:, :, None] == gathered[:, None, :]
    counts = eq.sum(axis=-1)
    score = counts.astype(np.int64) * (NUM_CLASSES + 1) - gathered
    idx = np.argmax(score, axis=1)
    return np.take_along_axis(gathered, idx[:, None], axis=1)[:, 0]


def _group_cols(g):
    """g: [rows, NC, n_groups] -> candidate columns [rows, NC*n_groups*G]."""
    rows = g.shape[0]
    cols = (
        g[:, :, :, None] * G
        + np.arange(G, dtype=np.int32)[None, None, None, :]
        + (np.arange(NC, dtype=np.int32) * S)[None, :, None, None]
    )
    return cols.reshape(rows, -1)


def host_finish(g_idx_all, d, labels):
    """g_idx_all: [NC, R, NSEL] selected group ids (last row-tile's ids
    are over groups [0, NG_LAST); its groups [NG_LAST, NG) were never
    streamed on device and are force-included here).
    Returns winning labels [R]."""
    g = g_idx_all.transpose(1, 0, 2).astype(np.int32)  # [R, NC, NSEL]
    lr = R - P
    out = np.empty(R, dtype=np.int64)
    out[:lr] = _vote(d[:lr], _group_cols(g[:lr]), labels)
    forced = np.broadcast_to(
        np.arange(NG_LAST, NG, dtype=np.int32), (P, NC, NG - NG_LAST)
    )
    g_last = np.concatenate([g[lr:], forced], axis=2)
    out[lr:] = _vote(d[lr:], _group_cols(g_last), labels)
    return out


def run_device(d, trace=False):
    if "nc" not in _CACHE:
        _CACHE["nc"] = build_nc()
    nc = _CACHE["nc"]
    in_maps = [
        {"d": np.ascontiguousarray(d[:, c * S : (c + 1) * S])} for c in range(NC)
    ]
    res = run_bass_kernel_spmd(nc, in_maps, list(range(NC)), trace=trace)
    g_idx_all = np.stack(
        [np.asarray(res.results[c]["gidx"]).astype(np.int64) for c in range(NC)]
    )
    return g_idx_all, res


def kernel(distances, labels):
    d = np.ascontiguousarray(np.asarray(distances, dtype=np.float32))
    lab = np.asarray(labels)
    g_idx_all, _ = run_device(d)
    out = host_finish(g_idx_all, d, lab.astype(np.int64))
    return out.astype(lab.dtype)


# revision 14
# speedup vs baseline: 1.0217x; 1.0137x over previous
"""Distributed kNN-classifier kernel for Trainium2 (8 NeuronCores).

Strategy (classic distributed kNN, column-sharded):
  - distances [2048, 100000] f32 are sharded along the prototype (column)
    dim: core c gets columns [c*12500, (c+1)*12500).  The last row-tile
    only streams its first 8000 columns; its remaining 45 groups are
    force-included as host candidates (bounded augmentation sized so the
    post-stream pipeline drain is fully hidden).
  - On device, a 3-engine pipeline per core keeps the DMA bus (the
    roofline resource: 102.4 MB/core at ~360 GB/s) 100% busy:
      SP    streams input chunks of [128, 2500] f32 (16 row-tiles x 5
            column-chunks) through an 8-slot SBUF ring, and ships the
            last row-tile's output ids.
      DVE   grouped-min-reduces each chunk (groups of 100 columns,
            negated f32 output) into 125 group minima per row-tile,
            then selects the 24 groups with the smallest minima via 3
            rounds of max8 / max_index / match_replace.  Aggregate DVE
            work is ~16.4 us per 17.8 us of tile DMA, so the DMA bus
            stays the bottleneck; only the last tile's selection sits in
            the pipeline drain.
      Act   DMAs each finished row-tile's 24 selected group ids
            [128, 24] u16 out, overlapped with the input stream (only
            the last tile's output, on SP, sits in the drain).
  - Host: each group id maps to 100 consecutive columns.  The 8*24
    groups per row (19200 candidate columns) are gathered from the
    input, reduced to the exact global top-16 by (value, column-index)
    lexicographic order (bit-exact vs jax.lax.top_k tie semantics),
    labels looked up, and the mode-with-smallest-label vote computed
    exactly as the reference does.

Exactness argument: an element of per-core rank r lives in a group whose
min is among the r smallest element values, hence among the r
lexicographically-smallest group minima; selecting 24 groups therefore
covers every element of per-core rank <= 24, which covers the global
top-16 plus any realistic tie multiplicity at the boundary.
"""

import os
import sys

import numpy as np

sys.path.insert(0, "/opt/trn_rl_repo")

import concourse.bass as bass
import concourse.mybir as mybir
from concourse.bass_utils import run_bass_kernel_spmd

R = 2048          # rows (batch)
N = 100000        # prototypes (columns)
NC = 8            # cores
S = N // NC       # 12500 columns per core
P = 128           # partitions
NT = R // P       # 16 row-tiles
W = 2500          # columns per DMA chunk
CH = S // W       # 5 chunks per row-tile
G = 100           # columns per group
NG = S // G       # 125 groups per row-tile
NSEL = 24         # groups selected per row per core (3 rounds of max8)
K = 16
NUM_CLASSES = 100

NBUFX = 8         # input-chunk ring slots (8 * 10 KB/partition)

# Per-tile chunk lists: (col_offset, width).  Groups are 100 consecutive
# columns regardless of chunking, so chunk boundaries only need to be
# multiples of 100.  The last tile only streams columns [0, 8000): its
# final NG-NG_LAST=45 groups are force-included as candidates on the
# host instead, and the streamed tail is split 300/200 — both so the
# DMA+reduce+select chain after the final transfer is as short as
# possible (it is the pipeline drain of the whole kernel).
_STD = [(c * W, W) for c in range(CH)]
_LAST = _STD[:3] + [(7500, 300), (7800, 200)]
CHUNKS = [
    (t, off, width)
    for t in range(NT)
    for (off, width) in (_LAST if t == NT - 1 else _STD)
]
NCH = len(CHUNKS)
S_LAST = 8000     # columns streamed for the last tile
NG_LAST = S_LAST // G  # 105 selectable groups for the last tile

_CACHE = {}


def build_nc():
    """Raw-Bass SPMD program. Engine pipeline:

    SP -(dma_sem)-> DVE reduce+select -(sel_sem)-> Act output-DMA.
    red_sem releases x-ring slots back to SP.  DVE write->read pairs
    need explicit drain()s (DVE writes retire ~8 pipe stages after the
    next instruction's reads issue; read->write pairs are safe).
    """
    nc = bass.Bass()
    din = nc.declare_dram_parameter("d", [R, S], mybir.dt.float32, isOutput=False)
    gout = nc.declare_dram_parameter("gidx", [R, NSEL], mybir.dt.uint16, isOutput=True)

    from contextlib import ExitStack

    with ExitStack() as ctx:
        x = ctx.enter_context(nc.sbuf_tensor([P, NBUFX * W], mybir.dt.float32))
        gneg = ctx.enter_context(nc.sbuf_tensor([P, 2 * NG], mybir.dt.float32))
        m8 = ctx.enter_context(nc.sbuf_tensor([P, 8], mybir.dt.float32))
        gidx_all = ctx.enter_context(
            nc.sbuf_tensor([P, NT * NSEL], mybir.dt.uint16)
        )
        # One DMA-completion semaphore per x-ring slot.  A dma_start's 16
        # increments are per-DMA-engine completions, so increments from
        # overlapping chunk DMAs interleave and a single cumulative
        # semaphore cannot tell "chunk k fully landed".  With one sem per
        # slot, chunk k and the next user of its sem (chunk k+NBUFX) are
        # serialized by the slot-reuse wait (SP only issues chunk k+NBUFX
        # after DVE consumed chunk k), so each wait is exact.
        dsem = [
            ctx.enter_context(nc.semaphore(f"dma_sem{j}")) for j in range(NBUFX)
        ]
        red_sem = ctx.enter_context(nc.semaphore("red_sem"))
        sel_sem = ctx.enter_context(nc.semaphore("sel_sem"))
        out_sem = ctx.enter_context(nc.semaphore("out_sem"))
        block = ctx.enter_context(nc.Block())

        @block.sync
        def _(sync):
            for k, (t, off, width) in enumerate(CHUNKS):
                if k >= NBUFX:
                    # slot's previous chunk fully consumed by DVE's reduce
                    sync.wait_ge(red_sem, k - NBUFX + 1)
                s = k % NBUFX
                sync.dma_start(
                    out=x[:, s * W : s * W + width],
                    in_=din[t * P : (t + 1) * P, off : off + width],
                ).then_inc(dsem[s], 16)
            # the last tile's output DMA: SP is idle once the input stream
            # is issued, and its DGE pipeline is ~240 ns shorter than Act's
            sync.wait_ge(sel_sem, NT)
            sync.dma_start(
                out=gout[(NT - 1) * P :, :],
                in_=gidx_all[:, (NT - 1) * NSEL :],
            ).then_inc(out_sem, 16)

        @block.scalar
        def _(scalar):
            for t in range(NT - 1):
                scalar.wait_ge(sel_sem, t + 1)
                scalar.dma_start(
                    out=gout[t * P : (t + 1) * P, :],
                    in_=gidx_all[:, t * NSEL : (t + 1) * NSEL],
                ).then_inc(out_sem, 16)
            scalar.wait_ge(out_sem, 16 * NT)

        @block.vector
        def _(vector):
            for k, (t, off, width) in enumerate(CHUNKS):
                par = t % 2
                s = k % NBUFX
                vector.wait_ge(dsem[s], 16 * (k // NBUFX + 1))
                # gneg[p, g] = -min over group = max over group of -x
                nc.vector.tensor_reduce(
                    out=gneg[
                        :, par * NG + off // G : par * NG + (off + width) // G
                    ],
                    in_=x[:, s * W : s * W + width].rearrange(
                        "p (g e) -> p g e", e=G
                    ),
                    axis=mybir.AxisListType.X,
                    op=mybir.AluOpType.min,
                    negate=True,
                ).then_inc(red_sem, 1)
                tile_end = S_LAST if t == NT - 1 else S
                if off + width == tile_end:
                    ng = NG_LAST if t == NT - 1 else NG
                    gv = gneg[:, par * NG : par * NG + ng]
                    nc.vector.drain()
                    for r in range(NSEL // 8):
                        nc.vector.max(out=m8[:], in_=gv)
                        nc.vector.drain()
                        nc.vector.max_index(
                            out=gidx_all[
                                :, t * NSEL + r * 8 : t * NSEL + (r + 1) * 8
                            ],
                            in_max=m8[:],
                            in_values=gv,
                        )
                        if r < NSEL // 8 - 1:
                            nc.vector.match_replace(
                                out=gv,
                                in_to_replace=m8[:],
                                in_values=gv,
                                imm_value=-3.0e38,
                            )
                            nc.vector.drain()
                    nc.vector.drain().then_inc(sel_sem, 1)

    return nc


def _sortable_u32(vals_f32):
    b = vals_f32.view(np.uint32)
    return np.where(b & 0x80000000, ~b, b | np.uint32(0x80000000)).astype(np.uint32)


def _vote(d_rows, cols, labels):
    """Exact top-K + mode vote for a row batch given candidate columns
    (cols must be duplicate-free per row)."""
    vals = np.take_along_axis(d_rows, cols, axis=1)
    key = (_sortable_u32(vals).astype(np.uint64) << np.uint64(17)) | cols.astype(
        np.uint64
    )
    key = np.partition(key, K - 1, axis=1)[:, :K]
    key.sort(axis=1)
    top_cols = (key[:, :K] & np.uint64(0x1FFFF)).astype(np.int64)
    gathered = labels[top_cols]  # [rows, K]
    eq = gathered[:, :, None] == gathered[:, None, :]
    counts = eq.sum(axis=-1)
    score = counts.astype(np.int64) * (NUM_CLASSES + 1) - gathered
    idx = np.argmax(score, axis=1)
    return np.take_along_axis(gathered, idx[:, None], axis=1)[:, 0]


def _group_cols(g):
    """g: [rows, NC, n_groups] -> candidate columns [rows, NC*n_groups*G]."""
    rows = g.shape[0]
    cols = (
        g[:, :, :, None] * G
        + np.arange(G, dtype=np.int32)[None, None, None, :]
        + (np.arange(NC, dtype=np.int32) * S)[None, :, None, None]
    )
    return cols.reshape(rows, -1)


def host_finish(g_idx_all, d, labels):
    """g_idx_all: [NC, R, NSEL] selected group ids (last row-tile's ids
    are over groups [0, NG_LAST); its groups [NG_LAST, NG) were never
    streamed on device and are force-included here).
    Returns winning labels [R]."""
    g = g_idx_all.transpose(1, 0, 2).astype(np.int32)  # [R, NC, NSEL]
    lr = R - P
    out = np.empty(R, dtype=np.int64)
    out[:lr] = _vote(d[:lr], _group_cols(g[:lr]), labels)
    forced = np.broadcast_to(
        np.arange(NG_LAST, NG, dtype=np.int32), (P, NC, NG - NG_LAST)
    )
    g_last = np.concatenate([g[lr:], forced], axis=2)
    out[lr:] = _vote(d[lr:], _group_cols(g_last), labels)
    return out


def run_device(d, trace=False):
    if "nc" not in _CACHE:
        _CACHE["nc"] = build_nc()
    nc = _CACHE["nc"]
    in_maps = [
        {"d": np.ascontiguousarray(d[:, c * S : (c + 1) * S])} for c in range(NC)
    ]
    res = run_bass_kernel_spmd(nc, in_maps, list(range(NC)), trace=trace)
    g_idx_all = np.stack(
        [np.asarray(res.results[c]["gidx"]).astype(np.int64) for c in range(NC)]
    )
    return g_idx_all, res


def kernel(distances, labels):
    d = np.ascontiguousarray(np.asarray(distances, dtype=np.float32))
    lab = np.asarray(labels)
    g_idx_all, _ = run_device(d)
    out = host_finish(g_idx_all, d, lab.astype(np.int64))
    return out.astype(lab.dtype)


# revision 16
# speedup vs baseline: 1.0291x; 1.0072x over previous
"""Distributed kNN-classifier kernel for Trainium2 (8 NeuronCores).

Strategy (classic distributed kNN, column-sharded):
  - distances [2048, 100000] f32 are sharded along the prototype (column)
    dim: core c gets columns [c*12500, (c+1)*12500).  The last row-tile
    only streams its first 6700 columns; its remaining 58 groups are
    force-included as host candidates (bounded augmentation sized so the
    post-stream pipeline drain is fully hidden).
  - On device, a 3-engine pipeline per core keeps the DMA bus (the
    roofline resource: 102.4 MB/core at ~360 GB/s) 100% busy:
      SP    streams input chunks of [128, 2500] f32 (16 row-tiles x 5
            column-chunks) through an 8-slot SBUF ring, and ships the
            last row-tile's output ids.
      DVE   grouped-min-reduces each chunk (groups of 100 columns,
            negated f32 output) into 125 group minima per row-tile,
            then selects the 24 groups with the smallest minima via 3
            rounds of max8 / max_index / match_replace.  Aggregate DVE
            work is ~16.4 us per 17.8 us of tile DMA, so the DMA bus
            stays the bottleneck; only the last tile's selection sits in
            the pipeline drain.
      Act   DMAs each finished row-tile's 24 selected group ids
            [128, 24] u16 out, overlapped with the input stream (only
            the last tile's output, on SP, sits in the drain).
  - Host: each group id maps to 100 consecutive columns.  The 8*24
    groups per row (19200 candidate columns) are gathered from the
    input, reduced to the exact global top-16 by (value, column-index)
    lexicographic order (bit-exact vs jax.lax.top_k tie semantics),
    labels looked up, and the mode-with-smallest-label vote computed
    exactly as the reference does.

Exactness argument: an element of per-core rank r lives in a group whose
min is among the r smallest element values, hence among the r
lexicographically-smallest group minima; selecting 24 groups therefore
covers every element of per-core rank <= 24, which covers the global
top-16 plus any realistic tie multiplicity at the boundary.
"""

import os
import sys

import numpy as np

sys.path.insert(0, "/opt/trn_rl_repo")

import concourse.bass as bass
import concourse.mybir as mybir
from concourse.bass_utils import run_bass_kernel_spmd

R = 2048          # rows (batch)
N = 100000        # prototypes (columns)
NC = 8            # cores
S = N // NC       # 12500 columns per core
P = 128           # partitions
NT = R // P       # 16 row-tiles
W = 2500          # columns per DMA chunk
CH = S // W       # 5 chunks per row-tile
G = 100           # columns per group
NG = S // G       # 125 groups per row-tile
NSEL = 24         # groups selected per row per core (3 rounds of max8)
K = 16
NUM_CLASSES = 100

NBUFX = 8         # input-chunk ring slots (8 * 10 KB/partition)

# Per-tile chunk lists: (col_offset, width).  Groups are 100 consecutive
# columns regardless of chunking, so chunk boundaries only need to be
# multiples of 100.  The last tile only streams columns [0, 6700): its
# final NG-NG_LAST=58 groups are force-included as candidates on the
# host instead, and the streamed tail is split 1200/300/200 — both so the
# DMA+reduce+select chain after the final transfer is as short as
# possible (it is the pipeline drain of the whole kernel).
_STD = [(c * W, W) for c in range(CH)]
_LAST = _STD[:2] + [(5000, 1200), (6200, 300), (6500, 200)]
CHUNKS = [
    (t, off, width)
    for t in range(NT)
    for (off, width) in (_LAST if t == NT - 1 else _STD)
]
NCH = len(CHUNKS)
S_LAST = 6700     # columns streamed for the last tile
NG_LAST = S_LAST // G  # 67 selectable groups for the last tile

_CACHE = {}


def build_nc():
    """Raw-Bass SPMD program. Engine pipeline:

    SP -(dma_sem)-> DVE reduce+select -(sel_sem)-> Act output-DMA.
    red_sem releases x-ring slots back to SP.  DVE write->read pairs
    need explicit drain()s (DVE writes retire ~8 pipe stages after the
    next instruction's reads issue; read->write pairs are safe).
    """
    nc = bass.Bass()
    din = nc.declare_dram_parameter("d", [R, S], mybir.dt.float32, isOutput=False)
    gout = nc.declare_dram_parameter("gidx", [R, NSEL], mybir.dt.uint16, isOutput=True)

    from contextlib import ExitStack

    with ExitStack() as ctx:
        x = ctx.enter_context(nc.sbuf_tensor([P, NBUFX * W], mybir.dt.float32))
        gneg = ctx.enter_context(nc.sbuf_tensor([P, 2 * NG], mybir.dt.float32))
        m8 = ctx.enter_context(nc.sbuf_tensor([P, 8], mybir.dt.float32))
        gidx_all = ctx.enter_context(
            nc.sbuf_tensor([P, NT * NSEL], mybir.dt.uint16)
        )
        # One DMA-completion semaphore per x-ring slot.  A dma_start's 16
        # increments are per-DMA-engine completions, so increments from
        # overlapping chunk DMAs interleave and a single cumulative
        # semaphore cannot tell "chunk k fully landed".  With one sem per
        # slot, chunk k and the next user of its sem (chunk k+NBUFX) are
        # serialized by the slot-reuse wait (SP only issues chunk k+NBUFX
        # after DVE consumed chunk k), so each wait is exact.
        dsem = [
            ctx.enter_context(nc.semaphore(f"dma_sem{j}")) for j in range(NBUFX)
        ]
        red_sem = ctx.enter_context(nc.semaphore("red_sem"))
        sel_sem = ctx.enter_context(nc.semaphore("sel_sem"))
        out_sem = ctx.enter_context(nc.semaphore("out_sem"))
        block = ctx.enter_context(nc.Block())

        @block.sync
        def _(sync):
            for k, (t, off, width) in enumerate(CHUNKS):
                if k >= NBUFX:
                    # slot's previous chunk fully consumed by DVE's reduce
                    sync.wait_ge(red_sem, k - NBUFX + 1)
                s = k % NBUFX
                sync.dma_start(
                    out=x[:, s * W : s * W + width],
                    in_=din[t * P : (t + 1) * P, off : off + width],
                ).then_inc(dsem[s], 16)
            # the last tile's output DMA: SP is idle once the input stream
            # is issued, and its DGE pipeline is ~240 ns shorter than Act's
            sync.wait_ge(sel_sem, NT)
            sync.dma_start(
                out=gout[(NT - 1) * P :, :],
                in_=gidx_all[:, (NT - 1) * NSEL :],
            ).then_inc(out_sem, 16)

        @block.scalar
        def _(scalar):
            for t in range(NT - 1):
                scalar.wait_ge(sel_sem, t + 1)
                scalar.dma_start(
                    out=gout[t * P : (t + 1) * P, :],
                    in_=gidx_all[:, t * NSEL : (t + 1) * NSEL],
                ).then_inc(out_sem, 16)
            scalar.wait_ge(out_sem, 16 * NT)

        @block.vector
        def _(vector):
            for k, (t, off, width) in enumerate(CHUNKS):
                par = t % 2
                s = k % NBUFX
                vector.wait_ge(dsem[s], 16 * (k // NBUFX + 1))
                # gneg[p, g] = -min over group = max over group of -x
                nc.vector.tensor_reduce(
                    out=gneg[
                        :, par * NG + off // G : par * NG + (off + width) // G
                    ],
                    in_=x[:, s * W : s * W + width].rearrange(
                        "p (g e) -> p g e", e=G
                    ),
                    axis=mybir.AxisListType.X,
                    op=mybir.AluOpType.min,
                    negate=True,
                ).then_inc(red_sem, 1)
                tile_end = S_LAST if t == NT - 1 else S
                if off + width == tile_end:
                    ng = NG_LAST if t == NT - 1 else NG
                    gv = gneg[:, par * NG : par * NG + ng]
                    nc.vector.drain()
                    for r in range(NSEL // 8):
                        nc.vector.max(out=m8[:], in_=gv)
                        nc.vector.drain()
                        nc.vector.max_index(
                            out=gidx_all[
                                :, t * NSEL + r * 8 : t * NSEL + (r + 1) * 8
                            ],
                            in_max=m8[:],
                            in_values=gv,
                        )
                        if r < NSEL // 8 - 1:
                            nc.vector.match_replace(
                                out=gv,
                                in_to_replace=m8[:],
                                in_values=gv,
                                imm_value=-3.0e38,
                            )
                            nc.vector.drain()
                    nc.vector.drain().then_inc(sel_sem, 1)

    return nc


def _sortable_u32(vals_f32):
    b = vals_f32.view(np.uint32)
    return np.where(b & 0x80000000, ~b, b | np.uint32(0x80000000)).astype(np.uint32)


def _vote(d_rows, cols, labels):
    """Exact top-K + mode vote for a row batch given candidate columns
    (cols must be duplicate-free per row)."""
    vals = np.take_along_axis(d_rows, cols, axis=1)
    key = (_sortable_u32(vals).astype(np.uint64) << np.uint64(17)) | cols.astype(
        np.uint64
    )
    key = np.partition(key, K - 1, axis=1)[:, :K]
    key.sort(axis=1)
    top_cols = (key[:, :K] & np.uint64(0x1FFFF)).astype(np.int64)
    gathered = labels[top_cols]  # [rows, K]
    eq = gathered[:, :, None] == gathered[:, None, :]
    counts = eq.sum(axis=-1)
    score = counts.astype(np.int64) * (NUM_CLASSES + 1) - gathered
    idx = np.argmax(score, axis=1)
    return np.take_along_axis(gathered, idx[:, None], axis=1)[:, 0]


def _group_cols(g):
    """g: [rows, NC, n_groups] -> candidate columns [rows, NC*n_groups*G]."""
    rows = g.shape[0]
    cols = (
        g[:, :, :, None] * G
        + np.arange(G, dtype=np.int32)[None, None, None, :]
        + (np.arange(NC, dtype=np.int32) * S)[None, :, None, None]
    )
    return cols.reshape(rows, -1)


def host_finish(g_idx_all, d, labels):
    """g_idx_all: [NC, R, NSEL] selected group ids (last row-tile's ids
    are over groups [0, NG_LAST); its groups [NG_LAST, NG) were never
    streamed on device and are force-included here).
    Returns winning labels [R]."""
    g = g_idx_all.transpose(1, 0, 2).astype(np.int32)  # [R, NC, NSEL]
    lr = R - P
    out = np.empty(R, dtype=np.int64)
    out[:lr] = _vote(d[:lr], _group_cols(g[:lr]), labels)
    forced = np.broadcast_to(
        np.arange(NG_LAST, NG, dtype=np.int32), (P, NC, NG - NG_LAST)
    )
    g_last = np.concatenate([g[lr:], forced], axis=2)
    out[lr:] = _vote(d[lr:], _group_cols(g_last), labels)
    return out


def run_device(d, trace=False):
    if "nc" not in _CACHE:
        _CACHE["nc"] = build_nc()
    nc = _CACHE["nc"]
    in_maps = [
        {"d": np.ascontiguousarray(d[:, c * S : (c + 1) * S])} for c in range(NC)
    ]
    res = run_bass_kernel_spmd(nc, in_maps, list(range(NC)), trace=trace)
    g_idx_all = np.stack(
        [np.asarray(res.results[c]["gidx"]).astype(np.int64) for c in range(NC)]
    )
    return g_idx_all, res


def kernel(distances, labels):
    d = np.ascontiguousarray(np.asarray(distances, dtype=np.float32))
    lab = np.asarray(labels)
    g_idx_all, _ = run_device(d)
    out = host_finish(g_idx_all, d, lab.astype(np.int64))
    return out.astype(lab.dtype)
